# revision 35
# baseline (speedup 1.0000x reference)
"""Multi-head attention (b=4, c=256, l=2048, 8 heads x 64) on 8 TRN2 NeuronCores.

Sharding: core i handles batch b = i//2 and query half qh = i%2 (1024 queries),
computing all 8 heads over the full 2048-key context. Outputs are disjoint
[256, 1024] slabs -> host-side concat only, no collectives.

Per-core kernel (all matmuls bf16, 1 cycle/row; fp32 PSUM accumulate):
  1. Q = Wq @ xq (1024 cols), K = Wk @ x (2048), VT = (Wv @ x)^T computed
     directly as x^T-stationary matmuls, laid out [l-tile 128, 8 heads x 65]
     with a ones column per head (col 64) for the softmax denominator.
  2. Per head h, per key-tile jt (16 x 128 keys):
       simT[j, i] = K_h(jt)^T . Q_h          (PSUM [128, 1024])
       E = exp(simT / 8)                     (ScalarE, PSUM -> SBUF)
       PV += VT'[jt, h]^T . E                (PSUM [65, 1024], accum over jt)
     Row 64 of PV = softmax denominator; rows 0..64 = numerator.
  3. recip = 1/PV[64] (DVE; last pair via exp(-ln) on ScalarE), partition-
     broadcast on GpSimd, attn = num * recip.
  4. out = WoutT^T . attn + bias, DMA to DRAM as bf16 (host upcasts).

Engine budget per core (measured): ScalarE exp stream ~143 us (saturated,
the roofline for this shape), PE ~168 us busy underneath it, DVE ~75 us,
phase-1 DMA lead-in ~20 us, tail+postamble ~16 us -> ~203 us total.
"""

import sys

if "/opt/trn_rl_repo" not in sys.path:
    sys.path.insert(0, "/opt/trn_rl_repo")

import numpy as np

import concourse.bass as bass
import concourse.mybir as mybir
import concourse.tile as tile
from concourse import bacc
from concourse.bass_utils import run_bass_kernel_spmd

F32 = mybir.dt.float32
F32R = mybir.dt.float32r
BF16 = mybir.dt.bfloat16
EXP = mybir.ActivationFunctionType.Exp
LOG = mybir.ActivationFunctionType.Ln
MULT = mybir.AluOpType.mult

B, C, L = 4, 256, 2048
H, D = 8, 64
HID = H * D  # 512
LQ = L // 2  # 1024 queries per core
NJT = L // 128  # 16 key tiles
SCALE = D**-0.5

_cached = {}


def r(ap):
    return ap


def build_nc():
    nc = bacc.Bacc(
        "TRN2",
        target_bir_lowering=False,
        debug=False,
        enable_asserts=False,
        num_devices=8,
    )
    x_d = nc.dram_tensor("x", [C, L], BF16, kind="ExternalInput")
    xq_d = nc.dram_tensor("xq", [C, LQ], BF16, kind="ExternalInput")
    wq_d = nc.dram_tensor("wqkvT", [C, 3 * HID], BF16, kind="ExternalInput")
    wo_d = nc.dram_tensor("woutT", [HID, C], BF16, kind="ExternalInput")
    bias_d = nc.dram_tensor("bias", [C, 1], F32, kind="ExternalInput")
    out_d = nc.dram_tensor("out", [C, LQ], BF16, kind="ExternalOutput")

    with tile.TileContext(nc) as tc:
        with (
            tc.tile_pool(name="const", bufs=1) as cp,
            tc.tile_pool(name="epool", bufs=6) as ep,
            tc.tile_pool(name="rpool", bufs=2) as rp,
            tc.tile_pool(name="opool", bufs=2) as op,
        ):
            # ---- persistent SBUF tensors ----
            xb = [cp.tile([128, L], BF16, tag=f"xb{k}", name=f"xb{k}") for k in range(2)]
            xq = [cp.tile([128, LQ], BF16, tag=f"xq{k}", name=f"xq{k}") for k in range(2)]
            wq = [cp.tile([128, 3 * HID], BF16, tag=f"wq{k}", name=f"wq{k}") for k in range(2)]
            wo = [cp.tile([128, C], BF16, tag=f"wo{k}", name=f"wo{k}") for k in range(4)]
            bias = [cp.tile([128, 1], F32, tag=f"bias{k}", name=f"bias{k}") for k in range(2)]
            Qs = [cp.tile([128, LQ], BF16, tag=f"Q{m}", name=f"Q{m}") for m in range(4)]
            Ks = [cp.tile([128, L], BF16, tag=f"K{m}", name=f"K{m}") for m in range(4)]
            VT = [cp.tile([128, H, D + 1], BF16, tag=f"VT{t}", name=f"VT{t}") for t in range(NJT)]
            attn = [cp.tile([128, LQ], BF16, tag=f"attn{m}", name=f"attn{m}") for m in range(4)]
            acc = [cp.tile([128, LQ], F32, tag=f"acc{m}", name=f"acc{m}") for m in range(2)]
            dum = cp.tile([1, 16], F32, tag="dum", name="dum")
            dumo = cp.tile([1, 16], F32, tag="dumo", name="dumo")
            nc.gpsimd.memset(dum[:], 1.0)
            nc.scalar.activation(dumo[:], dum[:], LOG)
            nc.scalar.activation(dumo[:], dum[:], EXP)

            # ---- DMA inputs ----
            nc.sync.dma_start(wq[0][:, 0:512], wq_d.ap()[0:128, 0:512])
            nc.gpsimd.dma_start(wq[1][:, 0:512], wq_d.ap()[128:256, 0:512])
            nc.sync.dma_start(wq[0][:, 512:640], wq_d.ap()[0:128, 512:640])
            nc.gpsimd.dma_start(wq[1][:, 512:640], wq_d.ap()[128:256, 512:640])
            for k in range(2):
                rows = slice(128 * k, 128 * (k + 1))
                nc.scalar.dma_start(xq[k][:, 0:512], xq_d.ap()[rows, 0:512])
            for k in range(2):
                rows = slice(128 * k, 128 * (k + 1))
                nc.scalar.dma_start(xq[k][:, 512:1024], xq_d.ap()[rows, 512:1024])
            for k in range(2):
                rows = slice(128 * k, 128 * (k + 1))
                nc.sync.dma_start(xb[k][:, 0:1024], x_d.ap()[rows, 0:1024])
            for k in range(2):
                rows = slice(128 * k, 128 * (k + 1))
                nc.scalar.dma_start(wq[k][:, 640:1024], wq_d.ap()[rows, 640:1024])
                nc.gpsimd.dma_start(xb[k][:, 1024:2048], x_d.ap()[rows, 1024:2048])
            for k in range(2):
                rows = slice(128 * k, 128 * (k + 1))
                nc.sync.dma_start(wq[k][:, 1024:1536], wq_d.ap()[rows, 1024:1536])
            for k in range(4):
                nc.sync.dma_start(wo[k][:], wo_d.ap()[128 * k : 128 * (k + 1), :])
            for k in range(2):
                rows = slice(128 * k, 128 * (k + 1))
                nc.gpsimd.dma_start(bias[k][:], bias_d.ap()[rows, :])

            # ---- phase 1: projections ----
            with (
                tc.tile_pool(name="pps", bufs=2, space=bass.MemorySpace.PSUM) as pps,
                tc.tile_pool(name="vps", bufs=2, space=bass.MemorySpace.PSUM) as vps,
            ):
                def q_proj(m):
                    ps = pps.tile([128, LQ], F32, tag="proj", name="ps")
                    for k in range(2):
                        for n in range(2):
                            nc.tensor.matmul(
                                ps[:, 512 * n : 512 * (n + 1)],
                                wq[k][:, 128 * m : 128 * (m + 1)],
                                xq[k][:, 512 * n : 512 * (n + 1)],
                                start=(k == 0),
                                stop=(k == 1),
                            )
                    if m == 0:
                        for n in range(2):
                            cols = slice(512 * n, 512 * (n + 1))
                            nc.vector.tensor_scalar_mul(
                                Qs[m][:, cols], ps[:, cols], SCALE
                            )
                    else:
                        nc.scalar.mul(Qs[m][:], ps[:], SCALE)

                def k_proj(m):
                    for lh in range(2):
                        ps = pps.tile([128, LQ], F32, tag="proj", name="ps")
                        for k in range(2):
                            for n in range(2):
                                nc.tensor.matmul(
                                    ps[:, 512 * n : 512 * (n + 1)],
                                    wq[k][:, HID + 128 * m : HID + 128 * (m + 1)],
                                    xb[k][:, 1024 * lh + 512 * n : 1024 * lh + 512 * (n + 1)],
                                    start=(k == 0),
                                    stop=(k == 1),
                                )
                        if m == 0:
                            for n in range(2):
                                nc.scalar.copy(
                                    Ks[m][
                                        :,
                                        1024 * lh + 512 * n : 1024 * lh + 512 * (n + 1),
                                    ],
                                    ps[:, 512 * n : 512 * (n + 1)],
                                )
                        else:
                            nc.vector.tensor_copy(
                                Ks[m][:, 1024 * lh : 1024 * (lh + 1)], ps[:]
                            )

                def vt_proj(t):
                    ps = vps.tile([128, HID], F32, tag="vproj", name="psv")
                    for k in range(2):
                        nc.tensor.matmul(
                            ps[:],
                            xb[k][:, 128 * t : 128 * (t + 1)],
                            wq[k][:, 2 * HID : 3 * HID],
                            start=(k == 0),
                            stop=(k == 1),
                        )
                    nc.vector.tensor_copy(
                        VT[t][:, :, 0:D], ps[:].rearrange("p (h c) -> p h c", h=H)
                    )
                    nc.gpsimd.memset(VT[t][:, :, D : D + 1], 1.0)

                q_proj(0)
                k_proj(0)
                vt_proj(0)
                vt_proj(1)
                q_proj(1)
                k_proj(1)
                vt_proj(2)
                vt_proj(3)
                q_proj(2)
                k_proj(2)
                q_proj(3)
                k_proj(3)
                for t in range(4, NJT):
                    vt_proj(t)

            # ---- phase 2: attention ----
            with (
                tc.tile_pool(name="qkps", bufs=2, space=bass.MemorySpace.PSUM) as qkps,
                tc.tile_pool(name="pvps", bufs=2, space=bass.MemorySpace.PSUM) as pvps,
            ):
                for p in range(4):
                    Qh = [Qs[p][64 * s : 64 * (s + 1), :] for s in (0, 1)]
                    Kh = [Ks[p][64 * s : 64 * (s + 1), :] for s in (0, 1)]
                    po = [
                        pvps.tile([D + 1, LQ], F32, tag="pv", name=f"po{s}")
                        for s in (0, 1)
                    ]
                    def pv(jt, s, E):
                        for n in range(2):
                            nc.tensor.matmul(
                                po[s][:, 512 * n : 512 * (n + 1)],
                                VT[jt][:, 2 * p + s, :],
                                E[:, 512 * n : 512 * (n + 1)],
                                start=(jt == 0),
                                stop=(jt == NJT - 1),
                            )

                    # software-pipelined: PV lags QK/exp by one iteration so
                    # the PE FIFO never blocks the next QK behind a PV whose
                    # exp hasn't finished
                    Eprev = None
                    for jt in range(NJT):
                        Es = []
                        for s in (0, 1):
                            psqk = qkps.tile(
                                [128, LQ], F32, tag="qk", name=f"psqk{s}"
                            )
                            nc.tensor.matmul(
                                psqk[:, 0:512],
                                Kh[s][:, 128 * jt : 128 * (jt + 1)],
                                Qh[s][:, 0:512],
                                start=True,
                                stop=True,
                            )
                            nc.tensor.matmul(
                                psqk[:, 512:1024],
                                Kh[s][:, 128 * jt : 128 * (jt + 1)],
                                Qh[s][:, 512:1024],
                                start=True,
                                stop=True,
                            )
                            if Eprev is not None:
                                pv(jt - 1, s, Eprev[s])
                            E = ep.tile([128, LQ], BF16, tag="e", name=f"E{s}")
                            nc.scalar.activation(E[:], psqk[:], EXP)
                            Es.append(E)
                        Eprev = Es
                    for s in (0, 1):
                        pv(NJT - 1, s, Eprev[s])
                    for s in (0, 1):
                        # copy numerator+denominator out of PSUM promptly so
                        # the PV psum slot frees for the next pair
                        rec = rp.tile([1, LQ], F32, tag="rec", name="rec")
                        rbc = rp.tile([64, LQ], F32, tag="rbc", name="rbc")
                        if p == 3:
                            # last pair: no next pair needs the PV psum slot,
                            # so normalize straight out of PSUM
                            lnd = rp.tile([1, LQ], F32, tag="lnd", name="lnd")
                            nc.scalar.activation(lnd[:], po[s][D : D + 1, :], LOG)
                            nc.scalar.activation(rec[:], lnd[:], EXP, scale=-1.0)
                            nc.gpsimd.partition_broadcast(rbc[:], rec[:])
                            nc.vector.tensor_tensor(
                                attn[p][64 * s : 64 * (s + 1), :],
                                po[s][0:D, :],
                                rbc[:],
                                MULT,
                            )
                        else:
                            pon = rp.tile(
                                [D + 1, LQ], F32, tag="pon", name="pon", bufs=4
                            )
                            nc.vector.tensor_copy(pon[:], po[s][:])
                            nc.vector.reciprocal(rec[:], pon[D : D + 1, :])
                            nc.gpsimd.partition_broadcast(rbc[:], rec[:])
                            nc.vector.tensor_tensor(
                                attn[p][64 * s : 64 * (s + 1), :],
                                pon[0:D, :],
                                rbc[:],
                                MULT,
                            )

            # ---- phase 3: output projection ----
            with tc.tile_pool(name="ops", bufs=2, space=bass.MemorySpace.PSUM) as ops:
                for m in range(2):
                    ps = ops.tile([128, LQ], F32, tag="o", name="pso")
                    for k in range(3):
                        for n in range(2):
                            nc.tensor.matmul(
                                ps[:, 512 * n : 512 * (n + 1)],
                                wo[k][:, 128 * m : 128 * (m + 1)],
                                attn[k][:, 512 * n : 512 * (n + 1)],
                                start=(k == 0),
                                stop=False,
                            )
                    for half in range(2):
                        hr = slice(64 * half, 64 * (half + 1))
                        for n in range(2):
                            nc.tensor.matmul(
                                ps[:, 512 * n : 512 * (n + 1)],
                                wo[3][hr, 128 * m : 128 * (m + 1)],
                                attn[3][hr, 512 * n : 512 * (n + 1)],
                                start=False,
                                stop=(half == 1),
                            )
                    osb = op.tile([128, LQ], BF16, tag="osb", name="osb")
                    for n in range(2):
                        cols = slice(512 * n, 512 * (n + 1))
                        if m == 0:
                            nc.scalar.add(osb[:, cols], ps[:, cols], bias[m][:])
                        else:
                            nc.vector.tensor_scalar_add(
                                osb[:, cols], ps[:, cols], bias[m][:]
                            )
                        nc.sync.dma_start(
                            out_d.ap()[128 * m : 128 * (m + 1), cols], osb[:, cols]
                        )

    nc.compile()
    return nc


def get_nc():
    if "nc" not in _cached:
        _cached["nc"] = build_nc()
    return _cached["nc"]


def make_in_maps(x, w_qkv, w_out, b_out):
    import ml_dtypes

    bf16 = ml_dtypes.bfloat16
    wqkvT = np.ascontiguousarray(w_qkv.T.astype(bf16))
    woutT = np.ascontiguousarray(w_out.T.astype(bf16))
    bias = np.ascontiguousarray(b_out.astype(np.float32).reshape(C, 1))
    in_maps = []
    for i in range(8):
        b, qh = i // 2, i % 2
        xb = np.ascontiguousarray(x[b].astype(bf16))
        xq = np.ascontiguousarray(xb[:, qh * LQ : (qh + 1) * LQ])
        in_maps.append(
            {"x": xb, "xq": xq, "wqkvT": wqkvT, "woutT": woutT, "bias": bias}
        )
    return in_maps


def assemble(results):
    out = np.empty((B, C, L), dtype=np.float32)
    for i in range(8):
        b, qh = i // 2, i % 2
        out[b][:, qh * LQ : (qh + 1) * LQ] = np.asarray(
            results[i]["out"], dtype=np.float32
        )
    return out


def kernel(x, w_qkv, w_out, b_out):
    x = np.asarray(x, dtype=np.float32)
    w_qkv = np.asarray(w_qkv, dtype=np.float32)
    w_out = np.asarray(w_out, dtype=np.float32)
    b_out = np.asarray(b_out, dtype=np.float32)
    assert x.shape == (B, C, L), x.shape
    nc = get_nc()
    in_maps = make_in_maps(x, w_qkv, w_out, b_out)
    res = run_bass_kernel_spmd(nc, in_maps, list(range(8)), trace=False)
    return assemble(res.results)
